# revision 1
# baseline (speedup 1.0000x reference)
"""Distributed Trainium2 kernel for AdaptiveGraphRecursiveConvolution.

Math (reference):
    out = relu( sum_g mix_w[g] * sum_k A_{gk} @ (h @ W[g,k])
              + sum_g inp_mix_w[g] * sum_k A_{gk} @ (x @ inp_W[g,k]) )

Folding the scalar mixing weights into the dense weights and merging the
h/x paths gives, with S = G*K edge sets and V_s = [mix_w*W_s ; inp_mix*inp_W_s]:
    out = relu( sum_s A_s @ (hx @ V_s) ),  hx = [h | x]  (N x 2F)

Device strategy (8 NeuronCores, SPMD single graph, per-core data):
  - dst-nodes sharded: core c owns rows [c*NSH, (c+1)*NSH).
  - Phase A (replicated): pre2 = hx @ [V_0..V_3]  ([NP2, S*128] bf16) via PE,
    streamed to private DRAM.
  - Phase B: per edge-set SpMM. Edges (pre-sharded/sorted/padded on host) are
    gathered per-edge from pre2 via SWDGE dma_gather (bf16 rows), and
    scatter-added into PSUM dst tiles via one-hot matmuls on PE:
        psum[dst_tile] += T_chunk^T @ msg_chunk
    where T[e, j] = val_e * (dst_loc_e == j) is built on DVE with a single
    fused tensor_scalar (is_equal then mult) per 128-edge subchunk.
  - relu on ACT from PSUM, DMA out. Host concatenates the 8 shards.
"""

import os
import sys
import time

import numpy as np

sys.path.insert(0, "/opt/trn_rl_repo")
sys.path.insert(0, "/root/.axon_site/_ro/trn_rl_repo")

import ml_dtypes  # noqa: E402

BF16 = ml_dtypes.bfloat16


# ---------------------------------------------------------------- config ---

class Cfg:
    def __init__(self, N, E, S=4, M=8, HALF=None, SUP=None, GCH=8, TGS=6, RB=12, NQ=4):
        self.N, self.E, self.S, self.M = N, E, S, M
        self.F2 = 256          # hx feature dim (2*128)
        self.O = 128
        self.NSH = N // M      # dst rows owned per core
        self.NPAD = _ru(self.NSH, 128)
        self.NT = self.NPAD // 128          # dst tiles per core
        self.TGS = TGS                      # tiles per PSUM group (<= 8 banks)
        self.TGN = -(-self.NT // TGS)       # tile groups
        self.HALF = HALF if HALF else _ru(-(-N // 2), 128)
        assert self.HALF <= 32767
        self.NP2 = 2 * self.HALF            # padded node rows in pre2
        assert N <= self.NP2
        self.SUP = SUP if SUP else 2560     # nodes per phase-A supertile
        assert self.SUP % 128 == 0 and self.NP2 % self.SUP == 0
        self.NSUP = self.NP2 // self.SUP
        self.T20 = self.SUP // 128
        self.NTA = self.NP2 // 128          # phase-A node tiles
        self.GCH = GCH                      # max subchunks (128 edges) per gather
        self.RB = RB                        # msg/T ring depth
        self.NQ = NQ                        # SWDGE queues


def _ru(x, m):
    return (x + m - 1) // m * m


FULL = Cfg(N=40000, E=640000, HALF=20480, SUP=2560)


# ------------------------------------------------------------- host prep ---

def _fold_weights(W, inp_W, mix_w, inp_mix_w, C):
    """Return v0, v1: [128, S*128] bf16 (h-path and x-path stationary weights)."""
    G, K = W.shape[0], W.shape[1]
    S = G * K
    Wm = (W.astype(np.float64) * mix_w.astype(np.float64)[:, None, None, None])
    Im = (inp_W.astype(np.float64) * inp_mix_w.astype(np.float64)[:, None, None, None])
    v0 = Wm.reshape(S, 128, 128).transpose(1, 0, 2).reshape(128, S * 128)
    v1 = Im.reshape(S, 128, 128).transpose(1, 0, 2).reshape(128, S * 128)
    return v0.astype(BF16), v1.astype(BF16)


def _prep_edges(edge_src, edge_dst, edge_val, C):
    """Shard/sort/pad edges. Returns (sched, per_core) where per_core[c] is a
    dict of wrapped device arrays and sched is the core-independent schedule.
    """
    S, E, M = C.S, C.E, C.M
    src = edge_src.reshape(S, E).astype(np.int64)
    dst = edge_dst.reshape(S, E).astype(np.int64)
    val = edge_val.reshape(S, E).astype(np.float32)

    # per (core, s, t, h): edge lists
    # raw[c][s][t][h] = (idx16, dstloc, val)
    # pre2 rows are PERMUTED within each half: node (p=sl%128, j=sl//128) is
    # stored at row p*NTH + j, so phase-A writes of 8 consecutive tiles give
    # each partition 8 KB of contiguous DRAM (big descriptors). Gather
    # indices are precomputed in permuted space; readiness class is by
    # j = idx % NTH (tiles are written in j order).
    NTH = C.HALF // 128
    raw = [[[[None, None] for _ in range(C.NT)] for _ in range(S)] for _ in range(M)]
    cnt = np.zeros((M, S, C.NT, 2), dtype=np.int64)
    for s in range(S):
        core_of = dst[s] // C.NSH
        for c in range(M):
            sel = np.nonzero(core_of == c)[0]
            d = dst[s][sel] - c * C.NSH
            t = d // 128
            h = (src[s][sel] >= C.HALF).astype(np.int64)
            # secondary sort by src so low-src subchunks can gather early
            key = (t * 2 + h) * (2 * C.HALF) + src[s][sel]
            order = np.argsort(key, kind="stable")
            sel, d, t, h = sel[order], d[order], t[order], h[order]
            key = key[order]
            bounds = np.searchsorted(key, np.arange(C.NT * 2 + 1) * (2 * C.HALF))
            for ti in range(C.NT):
                for hi in range(2):
                    a, b = bounds[ti * 2 + hi], bounds[ti * 2 + hi + 1]
                    ss = sel[a:b]
                    sl = src[s][ss] - hi * C.HALF
                    raw[c][s][ti][hi] = (
                        ((sl % 128) * NTH + sl // 128).astype(np.int16),
                        (d[a:b] - ti * 128).astype(np.int16),
                        val[s][ss],
                    )
                    cnt[c, s, ti, hi] = b - a

    # common padded lengths
    L = np.maximum(cnt.max(axis=0), 1)
    L = ((L + 127) // 128 * 128)  # [S, NT, 2]

    # schedule: segments in (tg, h, s) order (h outer so h=0 gathers can
    # start once the lower half of pre2 is written)
    segs = []     # dicts: s, h, tiles(list), n_sub, off (edges), off16, off128
    off = 0
    for tg in range(C.TGN):
        tiles = list(range(tg * C.TGS, min((tg + 1) * C.TGS, C.NT)))
        for h in range(2):
            for s in range(S):
                n_edges = int(sum(L[s][t][h] for t in tiles))
                segs.append(dict(tg=tg, s=s, h=h, tiles=tiles, n=n_edges,
                                 off=off))
                off += n_edges
    TOT = off
    assert TOT % 128 == 0

    # Per-(s,t,h) subchunk src-range class (PLVL levels per half): class e
    # means all cores' permuted indices have tile part (idx % NTH) <
    # (e+1)*NTH/PLVL, so its gather only needs that prefix of the half's
    # tiles to be written.
    PLVL = 4 if (C.NTA // 8) % 8 == 0 else 2
    QW = NTH // PLVL
    qcls = {}
    for s in range(S):
        for t in range(C.NT):
            for h in range(2):
                nsub = int(L[s][t][h]) // 128
                cls = []
                for j in range(nsub):
                    m = 0
                    for c in range(M):
                        seg_i = raw[c][s][t][h][0][j * 128:(j + 1) * 128]
                        if len(seg_i):
                            m = max(m, int((seg_i % NTH).max()))
                    cls.append(min(m // QW, PLVL - 1))
                qcls[(s, t, h)] = cls

    # subchunk -> (tile, slot) map, ordered low-quarter-first per segment
    sub_tile = []           # tile id per subchunk (global order)
    sub_q = []              # global quarter (2h + class) per subchunk
    sub_src = []            # (s, t, h, j) source slot per subchunk
    calls = []              # dict: seg index, sub_off (global), n_sub, s, h, q
    for gi, g in enumerate(segs):
        s, h = g["s"], g["h"]
        sub0 = g["off"] // 128
        ordered = []
        for t in g["tiles"]:
            for j in range(int(L[s][t][h]) // 128):
                ordered.append((qcls[(s, t, h)][j], t, j))
        ordered.sort()
        for cls, t, j in ordered:
            sub_tile.append(t)
            sub_q.append(PLVL * h + cls)
            sub_src.append((s, t, h, j))
        n_sub = len(ordered)
        o = 0
        while o < n_sub:
            take = min(C.GCH, n_sub - o)
            qlev = max(sub_q[sub0 + o: sub0 + o + take])
            calls.append(dict(seg=g, sub0=sub0 + o, n_sub=take, s=s, h=h,
                              q=qlev))
            o += take

    # Per-subchunk max real count over cores (for trailing-pad skip)
    def _nreal(sc):
        s, t, h, j = sc
        m = 0
        for c in range(M):
            m = max(m, min(128, max(0, len(raw[c][s][t][h][0]) - j * 128)))
        return m

    # within each call, move the subchunk with most skippable trailing pads
    # to the end; record the call's real (non-skipped) index count
    for cl in calls:
        a, b = cl["sub0"], cl["sub0"] + cl["n_sub"]
        pads = [128 - _nreal(sub_src[i]) for i in range(a, b)]
        kbest = int(np.argmax(pads))
        for arr in (sub_q, sub_src):
            arr[a + kbest:b] = arr[a + kbest + 1:b] + [arr[a + kbest]]
        st = list(sub_tile[a:b])
        st[kbest:] = st[kbest + 1:] + [st[kbest]]
        sub_tile[a:b] = st
        cl["nireg"] = cl["n_sub"] * 128 - pads[kbest]
    NSUB = len(sub_tile)

    # first/last subchunk per tile and the call index containing the last
    first_sub = np.full(C.NT, -1, dtype=np.int64)
    last_sub = np.full(C.NT, -1, dtype=np.int64)
    for i, t in enumerate(sub_tile):
        if first_sub[t] < 0:
            first_sub[t] = i
        last_sub[t] = i
    assert (first_sub >= 0).all()
    sub_call = np.zeros(NSUB, dtype=np.int64)
    for ci, cl in enumerate(calls):
        sub_call[cl["sub0"]: cl["sub0"] + cl["n_sub"]] = ci
    k_last = sub_call[last_sub]    # call index of each tile's last subchunk

    sched = dict(L=L, segs=segs, calls=calls, sub_tile=sub_tile, TOT=TOT,
                 NSUB=NSUB, first_sub=first_sub, last_sub=last_sub,
                 k_last=k_last, sub_call=sub_call, PLVL=PLVL)

    # per-core flattened arrays (slot layout follows sub_src permutation)
    per_core = []
    for c in range(M):
        idx = np.zeros(TOT, dtype=np.int16)
        dl = np.zeros(TOT, dtype=np.int64)
        vl = np.zeros(TOT, dtype=np.float32)
        for i, (s, t, h, j) in enumerate(sub_src):
            i16, d16, v32 = raw[c][s][t][h]
            a, b = j * 128, min((j + 1) * 128, len(i16))
            n = max(0, b - a)
            o = i * 128
            if n > 0:
                idx[o:o + n] = i16[a:b]
                dl[o:o + n] = d16[a:b]
                vl[o:o + n] = v32[a:b]
        for cl in calls:
            oe = (cl["sub0"] + cl["n_sub"]) * 128
            skip = cl["n_sub"] * 128 - cl["nireg"]
            if skip:
                idx[oe - skip:oe] = -1
        eidx = np.tile(idx.reshape(TOT // 16, 16).T, (8, 1))      # [128, TOT/16]
        # per-subchunk (dstloc, val) sidebands for on-chip one-hot T build:
        # T[e, i] = val_e * (i == dstloc_e), built on DVE via batched
        # tensor_tensor(is_equal) + tensor_tensor(mult) against an iota row.
        # Columns are CALL-ALIGNED (call b's subchunk k at column b*GCH+k,
        # pad val=0) so one DVE op can span several adjacent tb ring slots.
        NCALL = len(calls)
        dloc = np.zeros((128, NCALL * C.GCH), dtype=BF16)
        vals = np.zeros((128, NCALL * C.GCH), dtype=BF16)
        dl2 = dl.reshape(TOT // 128, 128).T.astype(np.float32)
        vl2 = vl.reshape(TOT // 128, 128).T
        for b, cl in enumerate(calls):
            a0, ns = cl["sub0"], cl["n_sub"]
            dloc[:, b * C.GCH:b * C.GCH + ns] = dl2[:, a0:a0 + ns].astype(BF16)
            vals[:, b * C.GCH:b * C.GCH + ns] = vl2[:, a0:a0 + ns].astype(BF16)
        per_core.append(dict(eidx=eidx, dloc=dloc, vals=vals))
    return sched, per_core


# ----------------------------------------------------------- graph build ---

def _build_graph(C, sched, mode="full"):
    import concourse.bass as bass
    import concourse.bacc as bacc
    import concourse.mybir as mybir
    from concourse.library_config import mlp
    from contextlib import ExitStack

    f32, bf16, i16 = mybir.dt.float32, mybir.dt.bfloat16, mybir.dt.int16
    S = C.S
    SW = S * 128                       # pre2 row width
    TOT = sched["TOT"]
    T16, T128 = TOT // 16, TOT // 128
    calls = sched["calls"]
    sub_tile = sched["sub_tile"]
    first_sub = set(sched["first_sub"].tolist())
    last_sub = set(sched["last_sub"].tolist())
    tile_of_first = {int(i): int(t) for t, i in enumerate(sched["first_sub"])}
    k_last = sched["k_last"]
    NSUB = sched["NSUB"]
    NCALL = len(calls)
    # cumulative subchunks per call (for PE waits on dve_sem)
    cum_subs = np.cumsum([0] + [cl["n_sub"] for cl in calls])

    sub_call = sched["sub_call"]
    TBLK = C.GCH                       # subchunks per T block == one call
    RBTB = 8                           # bf16 T ring
    ILV_CAP = 40                       # max early calls interleaved in phase A
    NB = NCALL                         # block b feeds call b
    GRP = 8                            # tiles per pre2 write group
    NG = C.NTA // GRP                  # pre2 write groups
    NTH = C.HALF // 128                # node tiles per half
    PLVL = sched["PLVL"]               # src-range classes per half
    NLV = 2 * PLVL                     # global levels
    QG = NG // NLV                     # groups per level
    assert NG % NLV == 0
    def _tot(qq, par):
        return sum(1 for y in range(qq * QG, (qq + 1) * QG) if y % 2 == par)

    nc = bacc.Bacc("TRN2", num_swdge_queues=C.NQ)
    hT = nc.declare_dram_parameter("hT", [128, C.NP2], bf16, isOutput=False)
    xT = nc.declare_dram_parameter("xT", [128, C.NP2], bf16, isOutput=False)
    v0d = nc.declare_dram_parameter("v0", [128, SW], bf16, isOutput=False)
    v1d = nc.declare_dram_parameter("v1", [128, SW], bf16, isOutput=False)
    TPAD = NCALL * C.GCH               # call-aligned subchunk columns
    eidxd = nc.declare_dram_parameter("eidx", [128, T16], i16, isOutput=False)
    dlocd = nc.declare_dram_parameter("dloc", [128, TPAD], bf16, isOutput=False)
    valsd = nc.declare_dram_parameter("vals", [128, TPAD], bf16, isOutput=False)
    iotad = nc.declare_dram_parameter("iota", [128, 128], bf16, isOutput=False)
    outd = nc.declare_dram_parameter("out", [C.NPAD, 128], f32, isOutput=True)
    pre2 = nc.dram_tensor("pre2", [C.NP2, SW], bf16)

    with ExitStack() as ctx:
        ec = ctx.enter_context
        # SBUF
        hx_sb = [[ec(nc.sbuf_tensor(f"hx{b}{k}", [128, C.SUP], bf16))
                  for k in range(2)] for b in range(2)]
        v_sb = [ec(nc.sbuf_tensor(f"v{k}_sb", [128, SW], bf16)) for k in range(2)]
        eidx_sb = ec(nc.sbuf_tensor("eidx_sb", [128, T16], i16))
        pout_sb = [ec(nc.sbuf_tensor(f"pout{b}", [128, GRP, SW], bf16)) for b in range(2)]
        msg_sb = [ec(nc.sbuf_tensor(f"msg{b}", [128, C.GCH, 128], bf16))
                  for b in range(C.RB)]
        tb_sb = ec(nc.sbuf_tensor("tb_sb", [128, RBTB * TBLK, 128], bf16))
        dloc_sb = ec(nc.sbuf_tensor("dloc_sb", [128, TPAD], bf16))
        vals_sb = ec(nc.sbuf_tensor("vals_sb", [128, TPAD], bf16))
        iota_sb = ec(nc.sbuf_tensor("iota_sb", [128, 128], bf16))
        outb_sb = [ec(nc.sbuf_tensor(f"ob{b}", [128, 128], f32)) for b in range(4)]
        # PSUM: 8 full banks
        psum = [ec(nc.psum_tensor(f"ps{b}", [128, 512], f32)) for b in range(8)]
        # semaphores
        in_sem = ec(nc.semaphore("in_sem"))
        hxs = [ec(nc.semaphore(f"hxs{i}")) for i in range(2)]
        p2wq = [[ec(nc.semaphore(f"p2wq{q}_{i}")) for i in range(2)]
                for q in range(NLV)]
        gths = [ec(nc.semaphore(f"gths{i}")) for i in range(C.RB)]
        dve_c = ec(nc.semaphore("dve_c"))
        outws = [ec(nc.semaphore(f"outws{i}")) for i in range(4)]
        mz = ec(nc.semaphore("mz"))
        mmA = ec(nc.semaphore("mmA"))
        cpy = ec(nc.semaphore("cpy"))
        peB = ec(nc.semaphore("peB"))
        rlu = ec(nc.semaphore("rlu"))

        with nc.Block() as block:

            @block.sync
            def _(sync):
                sync.dma_start(out=v_sb[0][:], in_=v0d[:]).then_inc(in_sem, 16)
                sync.dma_start(out=v_sb[1][:], in_=v1d[:]).then_inc(in_sem, 16)
                sync.dma_start(out=eidx_sb[:], in_=eidxd[:]).then_inc(in_sem, 16)
                sync.dma_start(out=dloc_sb[:], in_=dlocd[:]).then_inc(in_sem, 16)
                sync.dma_start(out=vals_sb[:], in_=valsd[:]).then_inc(in_sem, 16)
                sync.dma_start(out=iota_sb[:], in_=iotad[:]).then_inc(in_sem, 16)
                for sup in range(C.NSUP):
                    if sup >= 2:
                        # PE finished matmuls of super sup-2
                        sync.wait_ge(mmA, C.T20 * (sup - 1))
                    sl = slice(sup * C.SUP, (sup + 1) * C.SUP)
                    sync.dma_start(out=hx_sb[sup % 2][0][:],
                                   in_=hT[:, sl]).then_inc(hxs[sup % 2], 16)
                    sync.dma_start(out=hx_sb[sup % 2][1][:],
                                   in_=xT[:, sl]).then_inc(hxs[sup % 2], 16)

            # interleave schedule: which early calls run between phase-A
            # supers (quarter-gated so their pre2 data exists)
            ilv_sched = {}
            n_ilv = 0
            if mode == "full":
                ci = 0
                for sup in range(C.NSUP):
                    aq = ((sup - 1) * NLV) // C.NSUP - 1
                    if aq < 0:
                        continue
                    take = []
                    while (ci < min(ILV_CAP, NCALL) and len(take) < 4
                           and calls[ci]["q"] <= aq):
                        take.append(ci)
                        ci += 1
                    if take:
                        ilv_sched[sup] = take
                n_ilv = ci

            @block.tensor
            def _(tensor):
                state = dict(mz=False)

                def _emit_call(ci):
                    cl = calls[ci]
                    if not state["mz"]:
                        tensor.wait_ge(mz, C.RB)
                        state["mz"] = True
                    tensor.wait_ge(gths[ci % C.RB], 16 * (ci // C.RB + 1))
                    if mode == "A+gather":
                        tensor.nop().then_inc(peB)
                        return
                    tensor.wait_ge(dve_c, ci + 1)
                    for k in range(cl["n_sub"]):
                        i = cl["sub0"] + k
                        t = int(sub_tile[i])
                        slot = 2 + t % C.TGS
                        if i in first_sub and t >= C.TGS and mode != "B-norelu":
                            # previous occupant of this psum slot relu'd
                            tensor.wait_ge(rlu, t - C.TGS + 1)
                        mm = tensor.matmul(
                            psum[slot][:, 0:128],
                            tb_sb[:, (ci % RBTB) * TBLK + k, :],
                            msg_sb[ci % C.RB][:, k, :],
                            start=(i in first_sub), stop=(i in last_sub),
                            skip_group_check=True,
                        )
                        if k == cl["n_sub"] - 1:
                            mm.then_inc(peB)

                # ---- phase A (banks 0/1) with interleaved early calls ----
                tensor.wait_ge(in_sem, 96)
                for sup in range(C.NSUP):
                    tensor.wait_ge(hxs[sup % 2], 32 * (sup // 2 + 1))
                    for t20 in range(C.T20):
                        j = sup * C.T20 + t20
                        if j >= 2:
                            tensor.wait_ge(cpy, j - 1)
                        sl = slice(t20 * 128, (t20 + 1) * 128)
                        tensor.matmul(psum[j % 2][:, :], hx_sb[sup % 2][0][:, sl],
                                      v_sb[0][:], start=True, stop=False,
                                      skip_group_check=True)
                        tensor.matmul(psum[j % 2][:, :], hx_sb[sup % 2][1][:, sl],
                                      v_sb[1][:], start=False, stop=True,
                                      skip_group_check=True).then_inc(mmA)
                    for ci in ilv_sched.get(sup, []):
                        _emit_call(ci)
                # ---- phase B ----
                if mode == "A":
                    return
                for ci in range(n_ilv, NCALL):
                    _emit_call(ci)

            @block.scalar
            def _(scalar):
                Copy = mybir.ActivationFunctionType.Copy
                Relu = mybir.ActivationFunctionType.Relu
                # ---- phase A: psum -> bf16 sbuf -> pre2 dram ----
                # copies land in a wide pout buffer; one DMA per 4-tile group.
                # write-completion sems are per pre2 QUARTER so gathers can
                # start as soon as the quarter they read is written.
                for g in range(NG):
                    if g >= 2:
                        # pout buf g%2 free once group g-2's DMA done
                        qq2 = (g - 2) // QG
                        cnt = sum(1 for y in range(qq2 * QG,
                                                   min(g - 1, (qq2 + 1) * QG))
                                  if y % 2 == g % 2)
                        scalar.wait_ge(p2wq[qq2][g % 2], 16 * cnt)
                    for q in range(GRP):
                        j = GRP * g + q
                        scalar.wait_ge(mmA, j + 1)
                        scalar.activation(pout_sb[g % 2][:, q, :], psum[j % 2][:, :],
                                          Copy).then_inc(cpy)
                    scalar.wait_ge(cpy, GRP * (g + 1))
                    sem = p2wq[g // QG][g % 2]
                    # permuted half view: node (p, j) -> row p*NTH + j
                    hh = g // (NG // 2)
                    j0 = (g % (NG // 2)) * GRP
                    ph = pre2[hh * C.HALF:(hh + 1) * C.HALF, :].rearrange(
                        "(p j) c -> p j c", p=128)
                    scalar.dma_start(out=ph[:, j0:j0 + GRP, :],
                                     in_=pout_sb[g % 2][:, :, :]).then_inc(sem, 16)
                # ---- phase B: relu psum tiles -> out ----
                if mode not in ("full", "B-nodve"):
                    return
                r = 0
                nouts = [0, 0, 0, 0]
                for tg in range(C.TGN):
                    for t in range(tg * C.TGS, min((tg + 1) * C.TGS, C.NT)):
                        scalar.wait_ge(peB, int(k_last[t]) + 1)
                        if r >= 4:
                            scalar.wait_ge(outws[r % 4], 16 * (r // 4))
                        slot = 2 + t % C.TGS
                        scalar.activation(outb_sb[r % 4][:], psum[slot][:, 0:128],
                                          Relu).then_inc(rlu)
                        scalar.wait_ge(rlu, r + 1)
                        scalar.dma_start(out=outd[t * 128:(t + 1) * 128, :],
                                         in_=outb_sb[r % 4][:]).then_inc(outws[r % 4], 16)
                        nouts[r % 4] += 1
                        r += 1
                for q in range(4):
                    if nouts[q]:
                        scalar.wait_ge(outws[q], 16 * nouts[q])

            @block.vector
            def _(vector):
                if mode == "A":
                    return
                ieq, mul = mybir.AluOpType.is_equal, mybir.AluOpType.mult
                vector.wait_ge(in_sem, 96)
                for b in range(NB):
                    ns = calls[b]["n_sub"]
                    if b >= RBTB:
                        vector.wait_ge(peB, b - RBTB + 1)
                    s = (b % RBTB) * TBLK
                    tb = tb_sb[:, s:s + ns, :]
                    iota_b = iota_sb[:, :].unsqueeze(1).broadcast_to(
                        (128, ns, 128))
                    dloc_b = dloc_sb[:, b * TBLK:b * TBLK + ns].unsqueeze(
                        2).broadcast_to((128, ns, 128))
                    vals_b = vals_sb[:, b * TBLK:b * TBLK + ns].unsqueeze(
                        2).broadcast_to((128, ns, 128))
                    vector.tensor_tensor(tb, iota_b, dloc_b, ieq)
                    vector.tensor_tensor(tb, tb, vals_b, mul).then_inc(dve_c)

            @block.gpsimd
            def _(gpsimd):
                gpsimd.load_library(mlp)
                if mode == "A":
                    return
                gpsimd.wait_ge(in_sem, 96)
                for b in range(C.RB):
                    gpsimd.memzero(msg_sb[b][:]).then_inc(mz)
                gpsimd.wait_ge(mz, C.RB)
                nreg = nc.alloc_register(mybir.EngineType.Pool, "nidx")
                cur_q = -1
                for ci, cl in enumerate(calls):
                    s, h = cl["s"], cl["h"]
                    if cl["q"] > cur_q:
                        for qq in range(cur_q + 1, cl["q"] + 1):
                            gpsimd.wait_ge(p2wq[qq][0], 16 * _tot(qq, 0))
                            gpsimd.wait_ge(p2wq[qq][1], 16 * _tot(qq, 1))
                        cur_q = cl["q"]
                    if ci >= C.RB:
                        gpsimd.wait_ge(peB, ci - C.RB + 1)
                    n_idx = cl["n_sub"] * 128
                    src_ap = pre2[h * C.HALF:(h + 1) * C.HALF,
                                  s * 128:(s + 1) * 128]
                    o16 = cl["sub0"] * 8  # *128/16
                    gpsimd.reg_mov(nreg, cl["nireg"])
                    gpsimd.dma_gather(
                        msg_sb[ci % C.RB][:, 0:cl["n_sub"], :],
                        src_ap,
                        eidx_sb[:, o16:o16 + n_idx // 16],
                        n_idx, nreg, 128, elem_step=SW,
                        queue_num=ci % C.NQ,
                    ).then_inc(gths[ci % C.RB], 16)

    nc.compile()
    return nc


# ------------------------------------------------------------ entrypoint ---

_CACHE = {}


def _get_graph(C, sched_key, sched):
    if sched_key not in _CACHE:
        _CACHE[sched_key] = _build_graph(C, sched)
    return _CACHE[sched_key]


def _host_prep(h, x, W, inp_W, mix_w, inp_mix_w, edge_val, edge_src, edge_dst, C):
    v0, v1 = _fold_weights(np.asarray(W), np.asarray(inp_W),
                           np.asarray(mix_w), np.asarray(inp_mix_w), C)
    hTf = np.zeros((128, C.NP2), dtype=BF16)
    xTf = np.zeros((128, C.NP2), dtype=BF16)
    hTf[:, :C.N] = np.asarray(h, dtype=np.float32).T.astype(BF16)
    xTf[:, :C.N] = np.asarray(x, dtype=np.float32).T.astype(BF16)
    sched, per_core = _prep_edges(np.asarray(edge_src), np.asarray(edge_dst),
                                  np.asarray(edge_val, dtype=np.float32), C)
    iota = np.ascontiguousarray(
        np.broadcast_to(np.arange(128, dtype=np.float32)[None, :],
                        (128, 128))).astype(BF16)
    in_maps = []
    for c in range(C.M):
        in_maps.append(dict(
            hT=np.ascontiguousarray(hTf), xT=np.ascontiguousarray(xTf),
            v0=v0, v1=v1,
            eidx=np.ascontiguousarray(per_core[c]["eidx"]),
            dloc=per_core[c]["dloc"], vals=per_core[c]["vals"],
            iota=iota,
        ))
    return sched, in_maps


def kernel(h, x, W, inp_W, mix_w, inp_mix_w, edge_val, edge_src, edge_dst,
           _cfg=None, _trace=False):
    C = _cfg or FULL
    sched, in_maps = _host_prep(h, x, W, inp_W, mix_w, inp_mix_w,
                                edge_val, edge_src, edge_dst, C)
    key = (C.N, C.E, sched["TOT"], sched["NSUB"])
    nc = _get_graph(C, key, sched)

    from concourse.bass_utils import run_bass_kernel_spmd
    res = run_bass_kernel_spmd(nc, in_maps, core_ids=list(range(C.M)),
                               trace=_trace)
    out = np.empty((C.N, 128), dtype=np.float32)
    for c in range(C.M):
        out[c * C.NSH:(c + 1) * C.NSH] = res.results[c]["out"][:C.NSH]
    kernel._last_exec_ns = res.exec_time_ns
    return out

